# revision 1
# baseline (speedup 1.0000x reference)
# Multi-head attention (K/Q swapped variant) on 8 Trainium2 NeuronCores.
#
# Sharding: core = b*2 + half, b = batch (4), half = which 1024-row slice of
# the output sequence this core produces. Each core computes all 16 heads for
# its (batch, s-slice) and the final out-projection rows, so per-core outputs
# concatenate exactly into the full result (no cross-core reduction).
#
# Math (per batch b, head h), matching the reference exactly:
#   q[t] = (x[t] @ Wq.T + bq)/8 ; k[s] = x[s] @ Wk.T + bk
#   scoresT[t,s] = q[t] . k[s]        (= reference scores[s,t])
#   P[t,s] = exp(scoresT[t,s]) * mask[b,0,s,t]      (no max-subtraction:
#       scores are O(1) by construction; masked entries exact 0)
#   XP[d',s] = sum_t x_ext[t,d'] P[t,s]   (x_ext has a ones column, so
#       XP[64,s] = sum_t P[t,s] = softmax denominator)
#   outT_unnorm = wv_ext.T @ XP  (V-projection folded in after the attention
#       contraction by associativity; bias row rides the denominator)
#   out[s,e] = outT_unnorm[e,s] / denom[s];  y = outcat @ Wo.T + bo
import numpy as np

import concourse.bass as bass
import concourse.bacc as bacc
import concourse.mybir as mybir
import concourse.tile as tile
from concourse.bass_utils import run_bass_kernel_spmd

B, S, MD, NH, D = 4, 2048, 1024, 16, 64
SH = S // 2          # per-core output rows
TC = S // 128        # 16 t-chunks
F32 = mybir.dt.float32
F16 = mybir.dt.float16

_BUILD_CACHE = {}


def _build(loop_n=1):
    if loop_n in _BUILD_CACHE:
        return _BUILD_CACHE[loop_n]
    nc = bacc.Bacc("TRN2", target_bir_lowering=False, debug=False)

    xTq_d = nc.dram_tensor("xTq", [NH, D + 1, S], F16, kind="ExternalInput")
    xe_d = nc.dram_tensor("xe", [NH, 128, TC, D + 1], F16, kind="ExternalInput")
    mT_d = nc.dram_tensor("maskT", [128, TC, SH], F16, kind="ExternalInput")
    woT_d = nc.dram_tensor("woT", [MD, MD], F16, kind="ExternalInput")
    bo_d = nc.dram_tensor("bo", [1, MD], F16, kind="ExternalInput")
    wq_d = nc.dram_tensor("wq", [D + 1, D], F16, kind="ExternalInput")
    wk_d = nc.dram_tensor("wk", [D + 1, D], F16, kind="ExternalInput")
    wv_d = nc.dram_tensor("wv", [D + 1, D + 1], F16, kind="ExternalInput")
    y_d = nc.dram_tensor("y", [SH, MD], F32, kind="ExternalOutput")

    with tile.TileContext(nc) as tc:
        with tc.tile_pool(name="consts", bufs=1) as consts:
            woT_sb = consts.tile([128, 8, MD], F16, tag="wo")
            mT_sb = consts.tile([128, TC, SH], F16, tag="mT")
            for c in range(TC):
                nc.gpsimd.dma_start(out=mT_sb[:, c, :], in_=mT_d.ap()[:, c, :])
            wq_sb = consts.tile([D + 1, D], F16, tag="wq")
            nc.sync.dma_start(out=wq_sb[:], in_=wq_d.ap())
            wk_sb = consts.tile([D + 1, D], F16, tag="wk")
            nc.sync.dma_start(out=wk_sb[:], in_=wk_d.ap())
            wv_sb = consts.tile([D + 1, D + 1], F16, tag="wv")
            nc.sync.dma_start(out=wv_sb[:], in_=wv_d.ap())
            bo_sb = consts.tile([1, MD], F16, tag="bo")
            nc.sync.dma_start(out=bo_sb[:], in_=bo_d.ap())
            ones128 = consts.tile([1, 128], F16, tag="o128")
            nc.vector.memset(ones128[:], 1.0)
            occ = [consts.tile([128, SH], F16, tag=f"occ{c}", name=f"occ{c}") for c in range(8)]
            dn_g = [consts.tile([8, SH], F32, tag=f"dn{g}", name=f"dn{g}") for g in range(2)]
            rc_g = [consts.tile([8, SH], F32, tag=f"rc{g}", name=f"rc{g}") for g in range(2)]

            def body(_iv=None):
                with (
                    tc.tile_pool(name="xin", bufs=3) as xin,
                    tc.tile_pool(name="qk", bufs=3) as qk,
                    tc.tile_pool(name="pp", bufs=8) as pp,
                    tc.tile_pool(name="xps", bufs=3) as xps,
                    tc.tile_pool(name="dnst", bufs=3) as dnst,
                    tc.tile_pool(name="rbc", bufs=3) as rbc,
                    tc.tile_pool(name="dndr", bufs=4, space="DRAM") as dndr,
                    tc.tile_pool(name="mm", bufs=3, space="PSUM") as mm,
                    tc.tile_pool(name="acc", bufs=1, space="PSUM") as acc,
                ):

                    def emit_proj(h):
                        xTq_sb = xin.tile([D + 1, S], F16, tag="xq", name="xTq_sb")
                        for j in range(2):
                            nc.sync.dma_start(
                                out=xTq_sb[:, j * SH : (j + 1) * SH],
                                in_=xTq_d.ap()[h][:, j * SH : (j + 1) * SH],
                            )
                        qT_sb = qk.tile([D, S], F16, tag="q", name="qT_sb")
                        for j in range(2):
                            qp = mm.tile([D, SH], F32, tag="mm", name="qp")
                            for jj in (0, 512):
                                nc.tensor.matmul(
                                    qp[:, jj : jj + 512],
                                    wq_sb[:],
                                    xTq_sb[:, j * SH + jj : j * SH + jj + 512],
                                    start=True,
                                    stop=True,
                                )
                            nc.vector.tensor_copy(
                                qT_sb[:, j * SH : (j + 1) * SH], qp[:]
                            )
                        # k projection reads the local-half columns of xTq
                        kT_sb = qk.tile([D, SH], F16, tag="k", name="kT_sb")
                        kp = mm.tile([D, SH], F32, tag="mm", name="kp")
                        for jj in (0, 512):
                            nc.tensor.matmul(
                                kp[:, jj : jj + 512],
                                wk_sb[:],
                                xTq_sb[:, jj : jj + 512],
                                start=True,
                                stop=True,
                            )
                        nc.vector.tensor_copy(kT_sb[:], kp[:])
                        return qT_sb, kT_sb

                    qk_tiles = {0: emit_proj(0)}
                    sc0_next = None
                    for h in range(NH):
                        xe_sb = xin.tile([128, TC, D + 1], F16, tag="xe")
                        nc.sync.dma_start(out=xe_sb[:], in_=xe_d.ap()[h])
                        qT_sb, kT_sb = qk_tiles.pop(h)

                        # attention: scoresT -> exp -> mask -> XP accumulation
                        xp_ps = acc.tile([D + 1, SH], F32, tag="acc")

                        def emit_scores(c, q_t=qT_sb, k_t=kT_sb):
                            sc = mm.tile([128, SH], F32, tag="mm", name="sc")
                            for jj in (0, 512):
                                nc.tensor.matmul(
                                    sc[:, jj : jj + 512],
                                    q_t[:, c * 128 : (c + 1) * 128],
                                    k_t[:, jj : jj + 512],
                                    start=True,
                                    stop=True,
                                )
                            return sc

                        def emit_xp(c, pt):
                            for jj in (0, 512):
                                nc.tensor.matmul(
                                    xp_ps[:, jj : jj + 512],
                                    xe_sb[:, c, :],
                                    pt[:, jj : jj + 512],
                                    start=(c == 0),
                                    stop=(c == TC - 1),
                                )

                        if sc0_next is not None:
                            sc_tiles = {0: sc0_next[0]}
                            sc0_next = None
                        else:
                            sc_tiles = {0: emit_scores(0)}
                        pt_tiles = {}
                        for c in range(TC):
                            sc = sc_tiles.pop(c)
                            pt = pp.tile([128, SH], F16, tag="pt")
                            nc.scalar.activation(
                                pt[:], sc[:], mybir.ActivationFunctionType.Exp
                            )
                            if c + 1 < TC:
                                sc_tiles[c + 1] = emit_scores(c + 1)
                            nc.vector.tensor_mul(pt[:], pt[:], mT_sb[:, c, :])
                            pt_tiles[c] = pt
                            if c >= 1:
                                emit_xp(c - 1, pt_tiles.pop(c - 1))
                            if c == 5 and h + 1 < NH:
                                qk_tiles[h + 1] = emit_proj(h + 1)
                            if c == TC - 2 and h + 1 < NH:
                                nq, nk = qk_tiles[h + 1]
                                sc0_next = (emit_scores(0, nq, nk),)
                        emit_xp(TC - 1, pt_tiles.pop(TC - 1))
                        xp_sb = xps.tile([D + 1, SH], F16, tag="xps")
                        nc.vector.tensor_copy(xp_sb[:], xp_ps[:])
                        oT = acc.tile([D + 1, SH], F32, tag="acc")
                        for jj in (0, 512):
                            nc.tensor.matmul(
                                oT[:, jj : jj + 512],
                                wv_sb[:],
                                xp_sb[:, jj : jj + 512],
                                start=True,
                                stop=True,
                            )
                        c_idx, half = h // 2, h % 2
                        nc.vector.tensor_copy(
                            occ[c_idx][half * 64 : (half + 1) * 64, :], oT[0:64, :]
                        )
                        dnstage = dnst.tile([1, SH], F32, tag="dnst")
                        nc.vector.tensor_copy(dnstage[:], oT[64:65, :])
                        nc.sync.dma_start(out=dn_g[h // 8][h % 8 : h % 8 + 1, :], in_=dnstage[:])
                        if h % 8 == 7:
                            g = h // 8
                            nc.vector.reciprocal_approx_fast(
                                out=rc_g[g][:], in_=dn_g[g][:]
                            )
                            rc_dram = dndr.tile([8, SH], F16, tag="dndr")
                            nc.gpsimd.dma_start(out=rc_dram[:], in_=rc_g[g][:])
                            for ci in range(g * 4, (g + 1) * 4):
                                Rt = rbc.tile([128, SH], F16, tag="rbc")
                                for hf in range(2):
                                    rrow = rc_dram[(ci * 2 + hf) % 8 : (ci * 2 + hf) % 8 + 1, :]
                                    bcast = bass.AP(
                                        tensor=rrow.tensor,
                                        offset=rrow.offset,
                                        ap=[[0, 64]] + rrow.ap[1:],
                                    )
                                    nc.sync.dma_start(
                                        out=Rt[hf * 64 : (hf + 1) * 64, :], in_=bcast
                                    )
                                nc.vector.tensor_mul(
                                    occ[ci][:], occ[ci][:], Rt[:]
                                )

                for ec in range(8):
                    nc.gpsimd.dma_start(
                        out=woT_sb[:, ec, :],
                        in_=woT_d.ap().rearrange("(ec p) m -> p ec m", p=128)[:, ec, :],
                    )

                with (
                    tc.tile_pool(name="fin", bufs=2, space="PSUM") as fin,
                    tc.tile_pool(name="ysb", bufs=2) as ysb,
                ):
                    for si in range(8):
                        yp = fin.tile([128, MD], F32, tag="fin")
                        for jj in (0, 512):
                            for c_idx in range(8):
                                nc.tensor.matmul(
                                    yp[:, jj : jj + 512],
                                    occ[c_idx][:, si * 128 : (si + 1) * 128],
                                    woT_sb[:, c_idx, jj : jj + 512],
                                    start=(c_idx == 0),
                                    stop=False,
                                )
                            nc.tensor.matmul(
                                yp[:, jj : jj + 512],
                                ones128[:],
                                bo_sb[:, jj : jj + 512],
                                start=False,
                                stop=True,
                            )
                        y_sb = ysb.tile([128, MD], F32, tag="ysb")
                        nc.scalar.copy(y_sb[:], yp[:])
                        nc.sync.dma_start(
                            out=y_d.ap()[si * 128 : (si + 1) * 128, :], in_=y_sb[:]
                        )

            if loop_n > 1:
                with tc.For_i(0, loop_n, 1):
                    body()
            else:
                body()

    nc.compile()
    _BUILD_CACHE[loop_n] = nc
    return nc


def _prep(input, mask, Wk, bk, Wq, bq, Wv, bv, Wo, bo):
    x = np.ascontiguousarray(np.asarray(input, np.float32))
    mask = np.asarray(mask)
    f32 = np.float32

    wq_ext = np.concatenate(
        [np.asarray(Wq, f32).T, np.asarray(bq, f32)[None, :]], axis=0
    ) * f32(0.125)
    wk_ext = np.concatenate(
        [np.asarray(Wk, f32).T, np.asarray(bk, f32)[None, :]], axis=0
    )
    wv_ext = np.zeros((D + 1, D + 1), f32)
    wv_ext[:D, :D] = np.asarray(Wv, f32).T
    wv_ext[D, :D] = np.asarray(bv, f32)
    wv_ext[D, D] = 1.0
    woT = np.ascontiguousarray(np.asarray(Wo, f32).T)
    bo2 = np.asarray(bo, f32).reshape(1, MD)
    shared = {
        "wq": np.ascontiguousarray(wq_ext).astype(np.float16),
        "wk": np.ascontiguousarray(wk_ext).astype(np.float16),
        "wv": wv_ext.astype(np.float16),
        "woT": woT.astype(np.float16),
        "bo": bo2.astype(np.float16),
    }

    per_batch = []
    for b in range(B):
        xb = x[b]  # [S, MD]
        xTq = np.empty((NH, D + 1, S), np.float16)
        xTq[:, :D, :] = xb.T.reshape(NH, D, S)
        xTq[:, D, :] = 1.0
        xe = np.empty((NH, 128, TC, D + 1), np.float16)
        # [c,p,h,d] -> [h,p,c,d]
        xe[:, :, :, :D] = xb.reshape(TC, 128, NH, D).transpose(2, 1, 0, 3)
        xe[:, :, :, D] = 1.0
        per_batch.append((xTq, xe, np.asarray(mask[b, 0])))

    in_maps = []
    for core in range(8):
        b, half = core // 2, core % 2
        s0 = half * SH
        xTq, xe, mb = per_batch[b]
        # per-core t-permutation: local s-half chunks first
        if half == 0:
            xTq_p, xe_p = xTq, xe
        else:
            xTq_p = np.concatenate([xTq[:, :, SH:], xTq[:, :, :SH]], axis=2)
            xe_p = np.concatenate([xe[:, :, 8:, :], xe[:, :, :8, :]], axis=2)
        # maskT[p, c, sl] = mask[s0+sl, t(c)*128+p] with permuted t-chunk order
        mT = np.ascontiguousarray(
            mb[s0 : s0 + SH, :].reshape(SH, TC, 128).transpose(2, 1, 0)
        ).astype(np.float16)
        if half == 1:
            mT = np.ascontiguousarray(
                np.concatenate([mT[:, 8:, :], mT[:, :8, :]], axis=1)
            )
        in_maps.append(
            dict(
                shared,
                xTq=np.ascontiguousarray(xTq_p),
                xe=np.ascontiguousarray(xe_p),
                maskT=mT,
            )
        )
    return in_maps


def _assemble(results):
    y = np.empty((B, S, MD), np.float32)
    for core in range(8):
        b, half = core // 2, core % 2
        y[b, half * SH : (half + 1) * SH, :] = results[core]["y"]
    return y


def kernel(input, mask, Wk, bk, Wq, bq, Wv, bv, Wo, bo):
    in_maps = _prep(input, mask, Wk, bk, Wq, bq, Wv, bv, Wo, bo)
    nc = _build(1)
    res = run_bass_kernel_spmd(nc, in_maps, list(range(8)))
    return _assemble(res.results)


def timed_run(inputs, loop_n):
    """Run with the body repeated loop_n times on-device; returns wall seconds."""
    import time

    in_maps = _prep(**inputs)
    nc = _build(loop_n)
    t0 = time.perf_counter()
    res = run_bass_kernel_spmd(nc, in_maps, list(range(8)))
    t1 = time.perf_counter()
    return t1 - t0, _assemble(res.results)

